# revision 1
# baseline (speedup 1.0000x reference)
"""Trainium2 Bass kernel for nn_CrossAtt (dual cross-attention + 3x3 conv + BN + ReLU).

Sharding: 8 cores = (sample s in 0..3) x (h-half in 0..1). Each core computes
its 32 output rows plus a 1-row attention halo on each side (34 rows = 2176
query positions, host-zero-padded so the program is SPMD-uniform), then runs
the 3x3 conv locally. No collectives.

Device layout choices:
- scoresT [m, n] comes straight off the PE (lhsT=k zero-padded to K=128,
  rhs=q), so softmax needs no transpose of the 4096x2176 matrix.
- exp on ScalarE (no max subtraction; |scores| <~ 5 so fp32 exp is safe).
- AV: out^T[n, 257] = expT.T @ [vT | ones]; col 256 accumulates the softmax
  denominator S for free.
- normalize by (gamma * mask / S) as a per-partition scalar; mask zeroes the
  fake padded query rows. PE-transposes the small [n,256] result to [256,n].
- residual + gamma*bv bias are folded into the host-prepared x?r inputs.
- conv3x3 = 9 shifted matmuls over a [512, 35*66] zero-padded cat buffer;
  BN+ReLU fused into one activation (scale=inv, bias=beta per partition).
"""
import sys

if "/opt/trn_rl_repo" not in sys.path:
    sys.path.insert(0, "/opt/trn_rl_repo")

import numpy as np

import concourse.bass as bass
import concourse.bacc as bacc
import concourse.mybir as mybir
import concourse.tile as tile
from concourse.bass import ds, ts
from concourse.bass_utils import run_bass_kernel_spmd

F32 = mybir.dt.float32
F32R = mybir.dt.float32r  # same bits as fp32; 1 cycle/row PE mode (vs 4 for fp32)
EPS = 1e-5
P = 128
C = 256          # channels
M = 4096         # key/value positions (64*64)
NQ = 2176        # query positions per core (34 rows * 64), host padded
NROWS = 35       # cat_pad rows (34 data + 1 zero)
WPAD = 66        # cat_pad row width (64 + 2 zero cols)
# all moving dims >= 256 so the fp32r fast path applies
ATT_BLOCKS = [(i * 256, 256) for i in range(8)] + [(2048, 128)]
QWINS = [(0, 512), (512, 512), (1024, 512), (1536, 384), (1920, 256)]
CONV_WINS = [(1, 512), (513, 512), (1025, 512), (1537, 318), (1855, 256)]

_CACHE = {}


def _wins(total, w):
    return [(i, min(w, total - i)) for i in range(0, total, w)]


def _mm(nc, out, lhsT, rhs, **kw):
    nc.tensor.matmul(out, lhsT, rhs, **kw)


def _declare_io(nc):
    t = {}
    inp = lambda name, shape, dt=F32: t.__setitem__(
        name, nc.dram_tensor(name, shape, dt, kind="ExternalInput"))
    out = lambda name, shape, dt=F32: t.__setitem__(
        name, nc.dram_tensor(name, shape, dt, kind="ExternalOutput"))
    # fp32r = same 32-bit data; matmul operands must be declared fp32r end-to-end
    inp("x1", [C, M], F32R); inp("x2", [C, M], F32R)
    inp("x1q", [C, NQ], F32R); inp("x2q", [C, NQ], F32R)
    inp("x1r", [C, NQ]); inp("x2r", [C, NQ])
    inp("maskg", [P, 17])
    inp("wq1T", [P, 2, 32], F32R); inp("wq2T", [P, 2, 32], F32R)
    inp("wk1T", [P, 2, 64], F32R); inp("wk2T", [P, 2, 64], F32R)
    inp("wv1T", [P, 2, C], F32R); inp("wv2T", [P, 2, C], F32R)
    inp("bq1", [32, 1]); inp("bq2", [32, 1])
    inp("bk1", [64, 1]); inp("bk2", [64, 1])
    inp("cinv", [P, 2]); inp("cbeta", [P, 2])
    inp("wct", [3, 3, 2 * C, C], F32R)
    inp("ident", [P, P])
    out("feat", [C, 32, 64]); out("o1", [C, 32, 64], F32R); out("o2", [C, 32, 64], F32R)
    return t


def _emit(nc, tc, t, ctx):
    big = ctx.enter_context(tc.tile_pool(name="big", bufs=3))
    kqp = ctx.enter_context(tc.tile_pool(name="kq", bufs=1))
    sing = ctx.enter_context(tc.tile_pool(name="sing", bufs=1))
    expp = ctx.enter_context(tc.tile_pool(name="expp", bufs=3))
    normp = ctx.enter_context(tc.tile_pool(name="normp", bufs=3))
    scalp = ctx.enter_context(tc.tile_pool(name="scalp", bufs=4))
    resp = ctx.enter_context(tc.tile_pool(name="resp", bufs=4))
    wcp = ctx.enter_context(tc.tile_pool(name="wcp", bufs=12))
    psA = ctx.enter_context(tc.tile_pool(name="psA", bufs=2, space="PSUM"))
    psS = ctx.enter_context(tc.tile_pool(name="psS", bufs=2, space="PSUM"))

    BIG_SHAPE_BYTES = [P, 4 * NROWS * WPAD]  # cat_pad is the largest big tile

    # ---- constants / weights to SBUF ----
    idt = sing.tile([P, P], F32)
    nc.sync.dma_start(out=idt, in_=t["ident"][:])
    wq_sb, wk_sb, wv_sb, bq_sb, bk_sb = {}, {}, {}, {}, {}
    for b in (1, 2):
        wq_sb[b] = sing.tile([P, 2, 32], F32R, tag=f"wq{b}", name=f"wq{b}")
        nc.sync.dma_start(out=wq_sb[b], in_=t[f"wq{b}T"][:])
        wk_sb[b] = sing.tile([P, 2, 64], F32R, tag=f"wk{b}", name=f"wk{b}")
        nc.sync.dma_start(out=wk_sb[b], in_=t[f"wk{b}T"][:])
        wv_sb[b] = sing.tile([P, 2, C], F32R, tag=f"wv{b}", name=f"wv{b}")
        nc.sync.dma_start(out=wv_sb[b], in_=t[f"wv{b}T"][:])
        bq_sb[b] = sing.tile([32, 1], F32, tag=f"bq{b}", name=f"bq{b}")
        nc.sync.dma_start(out=bq_sb[b], in_=t[f"bq{b}"][:])
        bk_sb[b] = sing.tile([64, 1], F32, tag=f"bk{b}", name=f"bk{b}")
        nc.sync.dma_start(out=bk_sb[b], in_=t[f"bk{b}"][:])
    cinv_sb = sing.tile([P, 2], F32, tag="cinv")
    nc.sync.dma_start(out=cinv_sb, in_=t["cinv"][:])
    cbeta_sb = sing.tile([P, 2], F32, tag="cbeta")
    nc.sync.dma_start(out=cbeta_sb, in_=t["cbeta"][:])
    maskg_sb = sing.tile([P, 17], F32, tag="maskg")
    nc.sync.dma_start(out=maskg_sb, in_=t["maskg"][:])

    # ---- load x1, x2 (two column-half DMAs so the PE can start earlier) ----
    def load_x(name):
        x_sb = big.tile(BIG_SHAPE_BYTES, F32R, tag="big")
        xv = x_sb[:, : 2 * M].rearrange("p (kc n) -> p kc n", kc=2)
        src_ap = t[name][:].rearrange("(kc p) n -> p kc n", p=P)
        for c0 in range(0, M, 1024):
            nc.sync.dma_start(out=xv[:, :, ds(c0, 1024)],
                              in_=src_ap[:, :, ds(c0, 1024)])
        return xv

    x1_sb = load_x("x1")
    x2_sb = load_x("x2")

    # ---- k projections: k_b = wk_b @ x_b + bk_b, stored [128(c pad0), 4096] ----
    k_sb = {}
    for b, x_sb in ((1, x1_sb), (2, x2_sb)):
        kp = kqp.tile([P, M], F32R, tag=f"k{b}")
        for w0, ww in _wins(M, 512):
            ps = psS.tile([P, 1024], F32, tag="sc")
            for kc in range(2):
                _mm(nc, ps[0:64, :ww], wk_sb[b][:, kc, :],
                    x_sb[:, kc, ds(w0, ww)],
                    start=(kc == 0), stop=(kc == 1))
            nc.vector.tensor_scalar_add(kp[0:64, ds(w0, ww)], ps[0:64, :ww], bk_sb[b])
        k_sb[b] = kp

    # ---- vT projections: vT_b[m, c] = x_b.T @ wv_bT (no bias), plus ones col ----
    def make_vt(x_sb, b):
        vt = big.tile(BIG_SHAPE_BYTES, F32R, tag="big")
        vtv = vt[:, : 32 * 258].rearrange("p (mi c) -> p mi c", mi=32)
        nc.vector.memset(vtv[:, :, 256:257].bitcast(F32), 1.0)
        nc.vector.memset(vtv[:, :, 257:258].bitcast(F32), 0.0)
        for mi in range(32):
            ps_full = psS.tile([P, 1024], F32, tag="sc", name="vtps")
            ps = ps_full[:, :256]
            for kc in range(2):
                _mm(nc, ps, x_sb[:, kc, ts(mi, P)], wv_sb[b][:, kc, :],
                    start=(kc == 0), stop=(kc == 1))
            nc.vector.tensor_copy(out=vtv[:, mi, 0:256], in_=ps)
        return vtv

    # ---- q projection (shared by both branches): qp [128(c pad0), 2176] ----
    qp = kqp.tile([P, NQ], F32R, tag="qp")

    def q_half(name, b, row0):
        xq = big.tile(BIG_SHAPE_BYTES, F32R, tag="big")
        xqv = xq[:, : 2 * NQ].rearrange("p (kc n) -> p kc n", kc=2)
        xq_src = t[name][:].rearrange("(kc p) n -> p kc n", p=P)
        nc.sync.dma_start(out=xqv[:, :, 0:1088], in_=xq_src[:, :, 0:1088])
        nc.sync.dma_start(out=xqv[:, :, 1088:NQ], in_=xq_src[:, :, 1088:NQ])
        for w0, ww in QWINS:
            ps = psS.tile([P, 1024], F32, tag="sc")
            for kc in range(2):
                _mm(nc, ps[0:32, :ww], wq_sb[b][:, kc, :],
                    xqv[:, kc, ds(w0, ww)],
                    start=(kc == 0), stop=(kc == 1))
            nc.vector.tensor_scalar_add(qp[row0:row0 + 32, ds(w0, ww)],
                                        ps[0:32, :ww], bq_sb[b])

    q_half("x1q", 1, 0)
    vt1 = make_vt(x1_sb, 1)
    q_half("x2q", 2, 32)
    vt2 = make_vt(x2_sb, 2)

    # ---- cat_pad buffer [128, 4, 35*66], zeroed ----
    cat = big.tile(BIG_SHAPE_BYTES, F32R, tag="big")
    catv = cat[:].rearrange("p (i f) -> p i f", i=4)
    cat_r = cat[:].rearrange("p (i r w) -> p i r w", i=4, w=WPAD)
    nc.gpsimd.memset(cat[:].bitcast(F32), 0.0)

    # ---- attention branches ----
    for b, (kp, vtv, xr_name) in enumerate(
            [(k_sb[1], vt1, "x1r"), (k_sb[2], vt2, "x2r")]):
        for n0, nw in ATT_BLOCKS:
            nsub = nw // P
            g = 1024 // nw  # m-iters per exp group (4 for nw=256, 8 for 128)
            av = psA.tile([P, 1024], F32, tag="av")

            def flush_av(pend, av=av, vtv=vtv, nw=nw, nsub=nsub):
                g0, ex = pend
                for u in range(1024 // nw):
                    pmi = g0 + u
                    for j in range(nsub):
                        _mm(nc, av[:, ds(j * 512, 258)],
                            ex[:, ds(u * nw + j * P, P)], vtv[:, pmi, :],
                            start=(pmi == 0), stop=(pmi == 31))

            pend = None
            for g0 in range(0, 32, g):
                sc = psS.tile([P, 1024], F32, tag="sc")
                for u in range(g):
                    mi = g0 + u
                    _mm(nc, sc[:, ds(u * nw, nw)],
                        kp[0:64, ts(mi, P)], qp[0:64, ds(n0, nw)],
                        start=True, stop=True)
                ex = expp.tile([P, 1024], F32R, tag="ex")
                nc.scalar.activation(ex, sc, mybir.ActivationFunctionType.Exp)
                if pend is not None:
                    flush_av(pend)
                pend = (g0, ex)
            flush_av(pend)

            # epilogue per n-chunk of 128; transposes reuse the consumed AV bank
            for j in range(nsub):
                nch = n0 // P + j
                rs = scalp.tile([P, 1], F32, tag="rs")
                nc.vector.reciprocal(rs, av[:, ds(j * 512 + 256, 1)])
                nc.vector.tensor_mul(out=rs, in0=rs,
                                     in1=maskg_sb[:, ds(nch, 1)])
                nt = normp.tile([P, 256], F32, tag="nt")
                nc.vector.tensor_scalar_mul(nt, av[:, ds(j * 512, 256)], rs)
                rt = resp.tile([P, 2, P], F32, tag="rt")
                nc.sync.dma_start(
                    out=rt,
                    in_=t[xr_name][:].rearrange("(cc p) n -> p cc n", p=P)
                    [:, :, ts(nch, P)])
                for cc in range(2):
                    tp = av[:, ds(j * 512 + cc * P, P)]
                    nc.tensor.transpose(tp, nt[:, ts(cc, P)], idt)
                    nc.vector.tensor_add(
                        out=cat_r[:, 2 * b + cc, ds(2 * nch, 2), ds(1, 64)],
                        in0=tp.rearrange("p (r w) -> p r w", w=64),
                        in1=rt[:, cc, :].rearrange("p (r w) -> p r w", w=64))

        # write out this branch's attention output (rows 1..33 = the 32 real rows)
        ov = t[f"o{b + 1}"][:].rearrange("(cc p) h w -> p cc h w", p=P)
        for cc in range(2):
            nc.sync.dma_start(out=ov[:, cc],
                              in_=cat_r[:, 2 * b + cc, ds(1, 32), ds(1, 64)])

    # ---- conv 3x3 + BN + ReLU ----
    feat = big.tile(BIG_SHAPE_BYTES, F32, tag="big")
    featv = feat[:, : 2 * 2112].rearrange("p (o f) -> p o f", o=2)
    feat_r = feat[:, : 2 * 2112].rearrange("p (o r w) -> p o r w", o=2, w=WPAD)
    for oc in range(2):
        avc1 = psA.tile([P, 1024], F32, tag="av")
        avc2 = psA.tile([P, 1024], F32, tag="av")
        last = psS.tile([P, 1024], F32, tag="sc")

        def conv_dst(wi, ww, avc1=avc1, avc2=avc2, last=last):
            if wi < 2:
                return avc1[:, ds(wi * 512, ww)]
            if wi < 4:
                return avc2[:, ds((wi - 2) * 512, ww)]
            return last[:, :ww]

        wts = {}
        for ic in range(4):
            for tap in range(9):
                wt = wcp.tile([P, P], F32R, tag="wt", name=f"wt{oc}_{ic}_{tap}")
                nc.sync.dma_start(
                    out=wt, in_=t["wct"][tap // 3, tap % 3,
                                         ts(ic, P), ts(oc, P)])
                wts[(ic, tap)] = wt
        for ic in range(4):
            for tap in range(9):
                off = (tap // 3) * WPAD + (tap % 3) - 1
                for wi, (ws, ww) in enumerate(CONV_WINS):
                    _mm(nc, conv_dst(wi, ww), wts[(ic, tap)],
                        catv[:, ic, ds(ws + off, ww)],
                        start=(ic == 0 and tap == 0),
                        stop=(ic == 3 and tap == 8))
        for wi, (ws, ww) in enumerate(CONV_WINS):
            nc.scalar.activation(featv[:, oc, ds(ws, ww)], conv_dst(wi, ww),
                                 mybir.ActivationFunctionType.Relu,
                                 bias=cbeta_sb[:, ds(oc, 1)],
                                 scale=cinv_sb[:, ds(oc, 1)])
    fv = t["feat"][:].rearrange("(cc p) h w -> p cc h w", p=P)
    for oc in range(2):
        nc.sync.dma_start(out=fv[:, oc], in_=feat_r[:, oc, :, ds(1, 64)])


def _build():
    if "nc" in _CACHE:
        return _CACHE["nc"]
    nc = bacc.Bacc(None, target_bir_lowering=False)
    t = _declare_io(nc)
    from contextlib import ExitStack
    with tile.TileContext(nc) as tc, ExitStack() as ctx:
        _emit(nc, tc, t, ctx)
    nc.finalize()
    _CACHE["nc"] = nc
    return nc


def _prep_host(inputs):
    d = {k: np.ascontiguousarray(np.asarray(v, np.float32)) for k, v in inputs.items()}
    gamma = float(d["gamma"].reshape(-1)[0])
    inv = d["bn_scale"] / np.sqrt(d["bn_var"] + EPS)
    beta = d["bn_bias"] - d["bn_mean"] * inv

    def chunked(w):  # [256, o] -> [128, 2, o]
        return np.ascontiguousarray(w.reshape(2, P, -1).transpose(1, 0, 2))

    shared = {
        "wq1T": chunked(d["wq1"].T), "wq2T": chunked(d["wq2"].T),
        "wk1T": chunked(d["wk1"].T), "wk2T": chunked(d["wk2"].T),
        "wv1T": chunked(d["wv1"].T), "wv2T": chunked(d["wv2"].T),
        "bq1": d["bq1"].reshape(32, 1).copy(), "bq2": d["bq2"].reshape(32, 1).copy(),
        "bk1": d["bk1"].reshape(64, 1).copy(), "bk2": d["bk2"].reshape(64, 1).copy(),
        "cinv": np.ascontiguousarray(inv.reshape(2, P).T),
        "cbeta": np.ascontiguousarray(beta.reshape(2, P).T),
        "wct": np.ascontiguousarray(d["w_cat"].transpose(2, 3, 1, 0)),
        "ident": np.eye(P, dtype=np.float32),
    }
    gbv = {1: gamma * d["bv1"], 2: gamma * d["bv2"]}

    in_maps = []
    for core in range(8):
        s, half = core // 2, core % 2
        h0 = 32 * half
        x1 = np.ascontiguousarray(d["input1"][s].reshape(C, M))
        x2 = np.ascontiguousarray(d["input2"][s].reshape(C, M))
        n_lo, n_hi = (h0 - 1) * 64, (h0 + 33) * 64
        lo_pad, hi_pad = max(0, -n_lo), max(0, n_hi - M)
        sl = slice(n_lo + lo_pad, n_hi - hi_pad)

        def pad_slice(x, add=None):
            o = np.zeros((C, NQ), np.float32)
            body = x[:, sl]
            if add is not None:
                body = body + add[:, None]
            o[:, lo_pad:NQ - hi_pad] = body
            return o

        maskg = np.zeros(NQ, np.float32)
        maskg[lo_pad:NQ - hi_pad] = gamma
        m = dict(shared)
        m.update({
            "x1": x1, "x2": x2,
            "x1q": pad_slice(x1), "x2q": pad_slice(x2),
            "x1r": pad_slice(x1, gbv[1]), "x2r": pad_slice(x2, gbv[2]),
            "maskg": np.ascontiguousarray(maskg.reshape(17, P).T),
        })
        in_maps.append(m)
    return in_maps


def _run_cached_pjrt(nc, in_maps):
    """run_bass_via_pjrt equivalent with the traced/jitted executable cached
    across kernel() calls (run_bass_via_pjrt rebuilds it every call)."""
    import jax
    import numpy as _np
    from jax.sharding import Mesh, PartitionSpec
    from jax.experimental.shard_map import shard_map
    from concourse import bass2jax, mybir as _mb

    n_cores = len(in_maps)
    if "pjrt" not in _CACHE:
        bass2jax.install_neuronx_cc_hook()
        in_names, out_names, out_avals, zero_shapes = [], [], [], []
        for alloc in nc.m.functions[0].allocations:
            if not isinstance(alloc, _mb.MemoryLocationSet):
                continue
            name = alloc.memorylocations[0].name
            if alloc.kind == "ExternalInput":
                if nc.partition_id_tensor is None or \
                        name != nc.partition_id_tensor.name:
                    in_names.append(name)
            elif alloc.kind == "ExternalOutput":
                out_names.append(name)
                shape = tuple(alloc.tensor_shape)
                dtype = _mb.dt.np(alloc.dtype)
                out_avals.append(jax.core.ShapedArray(shape, dtype))
                zero_shapes.append((shape, dtype))
        n_params = len(in_names)
        all_names = in_names + out_names
        pid_name = nc.partition_id_tensor.name if nc.partition_id_tensor else None
        if pid_name is not None:
            all_names = all_names + [pid_name]

        def _body(*args):
            operands = list(args)
            if pid_name is not None:
                operands.append(bass2jax.partition_id_tensor())
            outs = bass2jax._bass_exec_p.bind(
                *operands,
                out_avals=tuple(out_avals),
                in_names=tuple(all_names),
                out_names=tuple(out_names),
                lowering_input_output_aliases=(),
                sim_require_finite=True,
                sim_require_nnan=True,
                nc=nc,
            )
            return tuple(outs)

        devices = jax.devices()[:n_cores]
        mesh = Mesh(_np.asarray(devices), ("core",))
        n_outs = len(out_names)
        sharded = jax.jit(
            shard_map(_body, mesh=mesh,
                      in_specs=(PartitionSpec("core"),) * (n_params + n_outs),
                      out_specs=(PartitionSpec("core"),) * n_outs,
                      check_rep=False),
            donate_argnums=tuple(range(n_params, n_params + n_outs)),
            keep_unused=True,
        )
        _CACHE["pjrt"] = (sharded, in_names, out_names, out_avals, zero_shapes)

    sharded, in_names, out_names, out_avals, zero_shapes = _CACHE["pjrt"]
    n_cores_ax = len(in_maps)
    concat_in = [
        _np.concatenate([_np.asarray(in_maps[c][nm]) for c in range(n_cores_ax)], axis=0)
        for nm in in_names
    ]
    concat_zeros = [
        _np.zeros((n_cores_ax * s[0], *s[1:]), d) for s, d in zero_shapes
    ]
    out_arrs = sharded(*concat_in, *concat_zeros)
    return [
        {nm: _np.asarray(out_arrs[i]).reshape(n_cores_ax, *out_avals[i].shape)[c]
         for i, nm in enumerate(out_names)}
        for c in range(n_cores_ax)
    ]


def kernel(**inputs):
    nc = _build()
    in_maps = _prep_host(inputs)
    try:
        results = _run_cached_pjrt(nc, in_maps)
    except Exception:
        _CACHE.pop("pjrt", None)
        res = run_bass_kernel_spmd(nc, in_maps, core_ids=list(range(8)))
        _CACHE["last_results"] = res
        results = res.results
    feat = np.zeros((4, C, 64, 64), np.float32)
    o1 = np.zeros((4, C, 64, 64), np.float32)
    o2 = np.zeros((4, C, 64, 64), np.float32)
    for core in range(8):
        s, half = core // 2, core % 2
        r = results[core]
        feat[s, :, 32 * half:32 * half + 32] = r["feat"]
        o1[s, :, 32 * half:32 * half + 32] = r["o1"]
        o2[s, :, 32 * half:32 * half + 32] = r["o2"]
    return (feat, o1, o2)



# revision 13
# speedup vs baseline: 1.6672x; 1.6672x over previous
"""Trainium2 Bass kernel for nn_CrossAtt (dual cross-attention + 3x3 conv + BN + ReLU).

Sharding: 8 cores = (sample s in 0..3) x (h-half in 0..1). Each core computes
its 32 output rows plus a 1-row attention halo on each side (34 rows = 2176
query positions, host-zero-padded so the program is SPMD-uniform), then runs
the 3x3 conv locally. No collectives.

Key speed choices (vs the fp32r baseline):
- All attention matmuls (q/k/v projections, scores, AV) run in fp8e4m3 with
  the DoubleRow perf mode: 0.5 PE cycles/row, 2x the fp32r rate. Host scales
  wq,wk by 64 and wv by 32 so the ~0.02-magnitude weights land in e4m3's
  normal range; the score scale is undone inside the exp activation
  (scale=1/4096) and the v scale inside the per-query normalizer.
- exp(scores) runs on the Act engine in [128,1024] tiles (PSUM->SBUF fp8),
  with bias=-2 so values stay in (5e-4, ~35) -- inside e4m3 range. The
  softmax denominator comes for free: each 128-channel vT chunk carries a
  ones column, so the DoubleRow AV accumulates S alongside the numerators.
- AV output is [n,c]-oriented; normalization (gamma*mask/(32*S)) is a
  per-partition scalar; results transpose back to [c,n] via bf16 PE
  transposes (1 cycle/row) into the cat buffer.
- The 3x3 conv stays fp32r ([128,264] windows = 4 output rows) but is
  interleaved into the attention stream so the PE fills the gaps while the
  Act engine (the bottleneck, ~140us of exp) streams softmax tiles.
- BN+ReLU runs on DVE (mult+add) + Pool (max 0), keeping Act exp-only.
- PSUM: 2x[128,1024] score tiles + 2x[128,512] AV tiles + 2x[128,512]
  conv/projection tiles = exactly 8 banks.
"""
import sys

if "/opt/trn_rl_repo" not in sys.path:
    sys.path.insert(0, "/opt/trn_rl_repo")

import numpy as np

import concourse.bass as bass
import concourse.bacc as bacc
import concourse.mybir as mybir
import concourse.tile as tile
from concourse.bass import ds, ts
from concourse.bass_utils import run_bass_kernel_spmd

F32 = mybir.dt.float32
F32R = mybir.dt.float32r
F8 = mybir.dt.float8e4
BF16 = mybir.dt.bfloat16
DR = mybir.MatmulPerfMode.DoubleRow
EPS = 1e-5
P = 128
C = 256          # channels
M = 4096         # key/value positions (64*64)
NQ = 2176        # query positions per core (34 rows * 64), host padded
NROWS = 35       # cat_pad rows (34 data + 1 zero)
WPAD = 66        # cat_pad row width (64 + 2 zero cols)
SQ = 64.0        # host scale on wq, wk (fp8 range); scores come out x4096
SV = 32.0        # host scale on wv
# n-superblocks: 8 x 256 queries + 1 x 128
SBS = [(i * 256, 256) for i in range(8)] + [(2048, 128)]

_CACHE = {}


def _declare_io(nc):
    t = {}
    inp = lambda name, shape, dt=F32: t.__setitem__(
        name, nc.dram_tensor(name, shape, dt, kind="ExternalInput"))
    out = lambda name, shape, dt=F32: t.__setitem__(
        name, nc.dram_tensor(name, shape, dt, kind="ExternalOutput"))
    inp("x1", [C, M], F8); inp("x2", [C, M], F8)
    inp("x1q", [C, NQ], F8); inp("x2q", [C, NQ], F8)
    inp("x1r", [C, NQ], BF16); inp("x2r", [C, NQ], BF16)
    inp("maskg", [P, 17])
    inp("wq1T", [P, 2, 32], F8); inp("wq2T", [P, 2, 32], F8)
    inp("wk1T", [P, 2, 64], F8); inp("wk2T", [P, 2, 64], F8)
    inp("wv1T", [P, 2, C], F8); inp("wv2T", [P, 2, C], F8)
    inp("bq1", [32, 1]); inp("bq2", [32, 1])
    inp("bk1", [64, 1]); inp("bk2", [64, 1])
    inp("cinv", [P, 2]); inp("cbeta", [P, 2])
    inp("wct", [3, 3, 2 * C, C], F32R)
    inp("ident16", [P, P], BF16)
    out("feat", [C, 32, 64]); out("o1", [C, 32, 64], F32R); out("o2", [C, 32, 64], F32R)
    return t


def _emit(nc, tc, t, ctx):
    big = ctx.enter_context(tc.tile_pool(name="big", bufs=2))
    sing = ctx.enter_context(tc.tile_pool(name="sing", bufs=1))
    kqp = ctx.enter_context(tc.tile_pool(name="kq", bufs=1))
    ktp = ctx.enter_context(tc.tile_pool(name="ktp", bufs=2))
    expp = ctx.enter_context(tc.tile_pool(name="expp", bufs=4))
    normp = ctx.enter_context(tc.tile_pool(name="normp", bufs=3))
    scalp = ctx.enter_context(tc.tile_pool(name="scalp", bufs=4))
    fwp = ctx.enter_context(tc.tile_pool(name="fwp", bufs=3))
    psS = ctx.enter_context(tc.tile_pool(name="psS", bufs=2, space="PSUM"))
    psA = ctx.enter_context(tc.tile_pool(name="psA", bufs=2, space="PSUM"))
    psM = ctx.enter_context(tc.tile_pool(name="psM", bufs=2, space="PSUM"))

    # ---- constants / weights to SBUF ----
    idt = sing.tile([P, P], BF16, tag="idt")
    nc.sync.dma_start(out=idt, in_=t["ident16"][:])
    ebias = sing.tile([P, 1], F32, tag="ebias")
    nc.vector.memset(ebias, -2.0)
    wq_sb, wk_sb, wv_sb, bq_sb, bk_sb = {}, {}, {}, {}, {}
    for b in (1, 2):
        wq_sb[b] = sing.tile([P, 2, 32], F8, tag=f"wq{b}", name=f"wq{b}")
        nc.sync.dma_start(out=wq_sb[b], in_=t[f"wq{b}T"][:])
        wk_sb[b] = sing.tile([P, 2, 64], F8, tag=f"wk{b}", name=f"wk{b}")
        nc.sync.dma_start(out=wk_sb[b], in_=t[f"wk{b}T"][:])
        wv_sb[b] = sing.tile([P, 2, C], F8, tag=f"wv{b}", name=f"wv{b}")
        nc.sync.dma_start(out=wv_sb[b], in_=t[f"wv{b}T"][:])
        bq_sb[b] = sing.tile([32, 1], F32, tag=f"bq{b}", name=f"bq{b}")
        nc.sync.dma_start(out=bq_sb[b], in_=t[f"bq{b}"][:])
        bk_sb[b] = sing.tile([64, 1], F32, tag=f"bk{b}", name=f"bk{b}")
        nc.sync.dma_start(out=bk_sb[b], in_=t[f"bk{b}"][:])
    cinv_sb = sing.tile([P, 2], F32, tag="cinv")
    nc.sync.dma_start(out=cinv_sb, in_=t["cinv"][:])
    cbeta_sb = sing.tile([P, 2], F32, tag="cbeta")
    nc.sync.dma_start(out=cbeta_sb, in_=t["cbeta"][:])
    maskg_sb = sing.tile([P, 17], F32, tag="maskg")
    nc.sync.dma_start(out=maskg_sb, in_=t["maskg"][:])

    # ---- big tiles: x8 pair (dies after projections), cat (lives to end) ----
    BIG_SHAPE = [P, 4 * NROWS * WPAD]  # 9240 f32 = 36.96KB/partition
    x8t = big.tile([P, 4 * M], F8, tag="big", name="x8pair")
    x8 = {}
    x8[1] = x8t[:, 0 * 2 * M: 1 * 2 * M].rearrange("p (kc n) -> p kc n", kc=2)
    x8[2] = x8t[:, 1 * 2 * M: 2 * 2 * M].rearrange("p (kc n) -> p kc n", kc=2)
    for b in (1, 2):
        src = t[f"x{b}"][:].rearrange("(kc p) n -> p kc n", p=P)
        for c0 in range(0, M, 2048):
            nc.sync.dma_start(out=x8[b][:, :, ds(c0, 2048)],
                              in_=src[:, :, ds(c0, 2048)])

    # query-slice inputs (fp8) and residuals (bf16)
    xq_sb, xr_sb = {}, {}
    for b in (1, 2):
        xq_sb[b] = kqp.tile([P, 2, NQ], F8, tag=f"xq{b}", name=f"xq{b}")
        nc.sync.dma_start(out=xq_sb[b],
                          in_=t[f"x{b}q"][:].rearrange("(kc p) n -> p kc n", p=P))
        xr_sb[b] = kqp.tile([P, 2, NQ], BF16, tag=f"xr{b}", name=f"xr{b}")
        nc.sync.dma_start(out=xr_sb[b],
                          in_=t[f"x{b}r"][:].rearrange("(cc p) n -> p cc n", p=P))

    # conv weights: one big strided DMA into [p, ic_chunk, tap, oc, 128]
    wct_sb = sing.tile([P, 4, 9, 2, P], F32R, tag="wct")
    wct_src = t["wct"][:].rearrange("t3a t3b (c p) (oc o) -> p c (t3a t3b) oc o",
                                    p=P, oc=2)
    for c in range(4):
        for oc in range(2):
            nc.sync.dma_start(out=wct_sb[:, c, :, oc, :],
                              in_=wct_src[:, c, :, oc, :])

    # ---- cat buffer [128, 4, 35*66] f32r, zeroed ----
    cat = big.tile(BIG_SHAPE, F32R, tag="big", name="cat")
    catv = cat[:].rearrange("p (i f) -> p i f", i=4)
    cat_r = cat[:].rearrange("p (i r w) -> p i r w", i=4, w=WPAD)
    nc.gpsimd.memset(cat[:].bitcast(F32), 0.0)

    # ---- projections (all fp8 DoubleRow) ----
    # q: shared [32, 2(branch), NQ]; pairing channel j = p + 32*i
    q8 = kqp.tile([32, 2, NQ], F8, tag="q8")
    for b in (1, 2):
        for w0 in range(0, NQ, 512):
            ww = min(512, NQ - w0)
            ps = psM.tile([P, 512], F32, tag="sm")
            for h in range(0, ww, 256):
                hw = min(256, ww - h)
                nc.tensor.matmul(ps[0:32, ds(h, hw)], wq_sb[b][:],
                                 xq_sb[b][:, :, ds(w0 + h, hw)],
                                 start=True, stop=True, perf_mode=DR)
            nc.vector.tensor_scalar_add(q8[:, b - 1, ds(w0, ww)],
                                        ps[0:32, :ww], bq_sb[b])

    # k: [64, M] then SBUF->SBUF DMA repack into [32, 2, M] pairing j = p + 32*i
    k8 = {}
    for b in (1, 2):
        ktmp = ktp.tile([P, M], F8, tag="ktmp")
        for w0 in range(0, M, 512):
            ps = psM.tile([P, 512], F32, tag="sm")
            for h in (0, 256):
                nc.tensor.matmul(ps[0:64, ds(h, 256)], wk_sb[b][:],
                                 x8[b][:, :, ds(w0 + h, 256)],
                                 start=True, stop=True, perf_mode=DR)
            nc.vector.tensor_scalar_add(ktmp[0:64, ds(w0, 512)],
                                        ps[0:64, :], bk_sb[b])
        kp = kqp.tile([32, 2, M], F8, tag=f"k8{b}", name=f"k8{b}")
        for i in range(2):
            nc.sync.dma_start(out=kp[:, i, :], in_=ktmp[ds(32 * i, 32), :])
        k8[b] = kp

    # vT: [128, 16(g), 2(i), 260]; per (g,i) row: [c0..127 | 1 | 0 | c128..255 | 1 | 0]
    vt = {}
    for b in (1, 2):
        vtt = kqp.tile([P, 16, 2, 260], F8, tag=f"vt{b}", name=f"vt{b}")
        for col, val in ((128, 1.0), (129, 0.0), (258, 1.0), (259, 0.0)):
            nc.gpsimd.memset(vtt[:, :, :, ds(col, 1)], val)
        vt[b] = vtt
        for g in range(16):
            ps = psM.tile([P, 512], F32, tag="sm")
            for i in range(2):
                mi = 2 * g + i
                nc.tensor.matmul(ps[:, ds(256 * i, 256)],
                                 x8[b][:, :, ts(mi, P)], wv_sb[b][:],
                                 start=True, stop=True, perf_mode=DR)
            # strided copy: psum (i, ch, 128) -> vt cols {0:128, 130:258} per i
            src = ps[:].rearrange("p (i ch c) -> p i ch c", i=2, ch=2)
            dst = vtt[:, g, :, :].rearrange("p i (ch rest) -> p i ch rest", ch=2)
            nc.vector.tensor_copy(out=dst[:, :, :, 0:128], in_=src)

    # ---- conv part scheduler ----
    # window k = output rows 4k..4k+3 (flat feat cols [264k, 264k+264));
    # ready after superblock si >= k+1 has been epilogued (needs j <= 2k+2).
    fv = t["feat"][:].rearrange("(cc p) h w -> p cc h w", p=P)
    conv_ps = {}   # (k, oc) -> psum tile
    conv_parts = [(k, oc, ic) for k in range(8) for oc in range(2)
                  for ic in range(4)]
    conv_emitted = 0

    def emit_conv_part(k, oc, ic):
        key = (k, oc)
        if key not in conv_ps:
            conv_ps[key] = psM.tile([P, 512], F32, tag="sm", name=f"cv{k}_{oc}")
        cps = conv_ps[key]
        # window k covers feat-flat [f0, f0+264); k=0 shifts by 1 so the
        # leftmost tap never reads offset -1 (f=0 is a junk pad column).
        sh = 1 if k == 0 else 0
        f0 = 264 * k + sh
        for tap in range(9):
            off = f0 + (tap // 3) * WPAD + (tap % 3) - 1
            nc.tensor.matmul(cps[:, 0:264], wct_sb[:, ic, tap, oc, :],
                             catv[:, ic, ds(off, 264)],
                             start=(ic == 0 and tap == 0),
                             stop=(ic == 3 and tap == 8))
        if ic == 3:
            fw = fwp.tile([P, 264], F32, tag="fw")
            nc.vector.tensor_scalar(fw, cps[:, 0:264],
                                    cinv_sb[:, ds(oc, 1)],
                                    cbeta_sb[:, ds(oc, 1)],
                                    op0=mybir.AluOpType.mult,
                                    op1=mybir.AluOpType.add)
            nc.gpsimd.tensor_scalar_max(fw, fw, 0.0)
            fwr = fw[:].rearrange("p (r w) -> p r w", w=WPAD)
            nc.sync.dma_start(out=fv[:, oc, ds(4 * k, 4), :],
                              in_=fwr[:, :, ds(1 - sh, 64)])
            del conv_ps[key]

    def conv_budget(done):
        # after `done` superblocks, j-chunks 0..2*done-1 are final; window k
        # reads cat rows 4k..4k+5 = j-chunks <= 2k+2, so need 2k+2 <= 2*done-1.
        if done >= len(SBS):
            return len(conv_parts)
        return 8 * max(0, done - 1)

    # ---- attention superblock loop ----
    ov = {b: t[f"o{b}"][:].rearrange("(cc p) h w -> p cc h w", p=P)
          for b in (1, 2)}

    for si, (n0, nw) in enumerate(SBS):
        for b in (1, 2):
            gpm = 1024 // nw          # mi-blocks per score tile (4 or 8)
            n_tiles = 32 // gpm       # score tiles in this superblock (8 or 4)
            nj = nw // P              # 128-wide n-chunks (2 or 1)
            av = [psA.tile([P, 512], F32, tag="av", name=f"av{si}_{b}_{j}")
                  for j in range(nj)]

            def flush_av(tkn, av=av, b=b, nj=nj, gpm=gpm, n_tiles=n_tiles):
                ex = ex_tiles[tkn]
                exv = ex[:].rearrange("p (g n) -> p g n", g=gpm)
                for j in range(nj):
                    for u2 in range(gpm // 2):
                        g = tkn * (gpm // 2) + u2
                        for ch in range(2):
                            nc.tensor.matmul(
                                av[j][:, ds(ch * 130, 130)],
                                exv[:, ds(2 * u2, 2), ds(j * P, P)],
                                vt[b][:, g, :, ds(ch * 130, 130)],
                                start=(tkn == 0 and u2 == 0),
                                stop=(tkn == n_tiles - 1 and u2 == gpm // 2 - 1),
                                perf_mode=DR)

            ex_tiles = {}
            pend = None
            for tkn in range(n_tiles):
                sc = psS.tile([P, 1024], F32, tag="sc")
                for u in range(gpm):
                    mi = tkn * gpm + u
                    nc.tensor.matmul(sc[:, ds(u * nw, nw)],
                                     k8[b][:, :, ts(mi, P)],
                                     q8[:, :, ds(n0, nw)],
                                     start=True, stop=True, perf_mode=DR)
                ex = expp.tile([P, 1024], F8, tag="ex")
                nc.scalar.activation(ex, sc, mybir.ActivationFunctionType.Exp,
                                     bias=ebias[:], scale=1.0 / 4096.0)
                ex_tiles[tkn] = ex
                if pend is not None:
                    flush_av(pend)
                pend = tkn
            flush_av(pend)

            # epilogue per 128-query chunk
            for j in range(nj):
                nch = n0 // P + j
                rs = scalp.tile([P, 1], F32, tag="rs")
                nc.vector.reciprocal(rs, av[j][:, ds(128, 1)])
                nc.vector.tensor_mul(out=rs, in0=rs, in1=maskg_sb[:, ds(nch, 1)])
                nt = normp.tile([P, 2, P], BF16, tag="nt")
                avv = av[j][:, 0:260].rearrange("p (ch rest) -> p ch rest", ch=2)
                nc.vector.tensor_scalar_mul(nt, avv[:, :, 0:128], rs)
                tp = av[j][:, 320:448].bitcast(BF16)
                for cc in range(2):
                    nc.tensor.transpose(tp[:, ts(cc, P)], nt[:, cc, :], idt)
                tpv = tp[:].rearrange("p (cc r w) -> p cc r w", cc=2, w=64)
                xrv = xr_sb[b][:, :, ds(P * nch, P)].rearrange(
                    "p cc (r w) -> p cc r w", w=64)
                nc.vector.tensor_add(
                    out=cat_r[:, ds(2 * (b - 1), 2), ds(2 * nch, 2), ds(1, 64)],
                    in0=tpv, in1=xrv)
                # stream out finished o1/o2 rows (cat rows 1..32 are real)
                r0, r1 = max(1, 2 * nch), min(32, 2 * nch + 1)
                if r1 >= r0:
                    nrr = r1 - r0 + 1
                    for cc in range(2):
                        nc.sync.dma_start(
                            out=ov[b][:, cc, ds(r0 - 1, nrr), :],
                            in_=cat_r[:, 2 * (b - 1) + cc, ds(r0, nrr), ds(1, 64)])

        # interleave conv windows that are now unlocked
        budget = conv_budget(si + 1)
        while conv_emitted < budget:
            emit_conv_part(*conv_parts[conv_emitted])
            conv_emitted += 1


def _build():
    if "nc" in _CACHE:
        return _CACHE["nc"]
    nc = bacc.Bacc(None, target_bir_lowering=False)
    t = _declare_io(nc)
    from contextlib import ExitStack
    with tile.TileContext(nc) as tc, ExitStack() as ctx:
        _emit(nc, tc, t, ctx)
    nc.finalize()
    _CACHE["nc"] = nc
    return nc


def _prep_host(inputs):
    import ml_dtypes
    E4 = ml_dtypes.float8_e4m3
    B16 = ml_dtypes.bfloat16
    d = {k: np.ascontiguousarray(np.asarray(v, np.float32)) for k, v in inputs.items()}
    gamma = float(d["gamma"].reshape(-1)[0])
    inv = d["bn_scale"] / np.sqrt(d["bn_var"] + EPS)
    beta = d["bn_bias"] - d["bn_mean"] * inv

    def chunked(w, scale):  # [256, o] -> [128, 2, o] fp8
        return np.ascontiguousarray(
            (w * scale).reshape(2, P, -1).transpose(1, 0, 2)).astype(E4)

    shared = {
        "wq1T": chunked(d["wq1"].T, SQ), "wq2T": chunked(d["wq2"].T, SQ),
        "wk1T": chunked(d["wk1"].T, SQ), "wk2T": chunked(d["wk2"].T, SQ),
        "wv1T": chunked(d["wv1"].T, SV), "wv2T": chunked(d["wv2"].T, SV),
        "bq1": (SQ * d["bq1"]).reshape(32, 1).copy(),
        "bq2": (SQ * d["bq2"]).reshape(32, 1).copy(),
        "bk1": (SQ * d["bk1"]).reshape(64, 1).copy(),
        "bk2": (SQ * d["bk2"]).reshape(64, 1).copy(),
        "cinv": np.ascontiguousarray(inv.reshape(2, P).T),
        "cbeta": np.ascontiguousarray(beta.reshape(2, P).T),
        "wct": np.ascontiguousarray(d["w_cat"].transpose(2, 3, 1, 0)),
        "ident16": np.eye(P, dtype=np.float32).astype(B16),
    }
    gbv = {1: gamma * d["bv1"], 2: gamma * d["bv2"]}

    in_maps = []
    for core in range(8):
        s, half = core // 2, core % 2
        h0 = 32 * half
        x1 = np.ascontiguousarray(d["input1"][s].reshape(C, M))
        x2 = np.ascontiguousarray(d["input2"][s].reshape(C, M))
        n_lo, n_hi = (h0 - 1) * 64, (h0 + 33) * 64
        lo_pad, hi_pad = max(0, -n_lo), max(0, n_hi - M)
        sl = slice(n_lo + lo_pad, n_hi - hi_pad)

        def pad_slice(x, add=None):
            o = np.zeros((C, NQ), np.float32)
            body = x[:, sl]
            if add is not None:
                body = body + add[:, None]
            o[:, lo_pad:NQ - hi_pad] = body
            return o

        maskg = np.zeros(NQ, np.float32)
        maskg[lo_pad:NQ - hi_pad] = gamma / SV
        m = dict(shared)
        m.update({
            "x1": x1.astype(E4), "x2": x2.astype(E4),
            "x1q": pad_slice(x1).astype(E4), "x2q": pad_slice(x2).astype(E4),
            "x1r": pad_slice(x1, gbv[1]).astype(B16),
            "x2r": pad_slice(x2, gbv[2]).astype(B16),
            "maskg": np.ascontiguousarray(maskg.reshape(17, P).T),
        })
        in_maps.append(m)
    return in_maps


def _run_cached_pjrt(nc, in_maps):
    """run_bass_via_pjrt equivalent with the traced/jitted executable cached
    across kernel() calls (run_bass_via_pjrt rebuilds it every call)."""
    import jax
    import numpy as _np
    from jax.sharding import Mesh, PartitionSpec
    from jax.experimental.shard_map import shard_map
    from concourse import bass2jax, mybir as _mb

    n_cores = len(in_maps)
    if "pjrt" not in _CACHE:
        bass2jax.install_neuronx_cc_hook()
        in_names, out_names, out_avals, zero_shapes = [], [], [], []
        for alloc in nc.m.functions[0].allocations:
            if not isinstance(alloc, _mb.MemoryLocationSet):
                continue
            name = alloc.memorylocations[0].name
            if alloc.kind == "ExternalInput":
                if nc.partition_id_tensor is None or \
                        name != nc.partition_id_tensor.name:
                    in_names.append(name)
            elif alloc.kind == "ExternalOutput":
                out_names.append(name)
                shape = tuple(alloc.tensor_shape)
                dtype = _mb.dt.np(alloc.dtype)
                out_avals.append(jax.core.ShapedArray(shape, dtype))
                zero_shapes.append((shape, dtype))
        n_params = len(in_names)
        all_names = in_names + out_names
        pid_name = nc.partition_id_tensor.name if nc.partition_id_tensor else None
        if pid_name is not None:
            all_names = all_names + [pid_name]

        def _body(*args):
            operands = list(args)
            if pid_name is not None:
                operands.append(bass2jax.partition_id_tensor())
            outs = bass2jax._bass_exec_p.bind(
                *operands,
                out_avals=tuple(out_avals),
                in_names=tuple(all_names),
                out_names=tuple(out_names),
                lowering_input_output_aliases=(),
                sim_require_finite=True,
                sim_require_nnan=True,
                nc=nc,
            )
            return tuple(outs)

        devices = jax.devices()[:n_cores]
        mesh = Mesh(_np.asarray(devices), ("core",))
        n_outs = len(out_names)
        sharded = jax.jit(
            shard_map(_body, mesh=mesh,
                      in_specs=(PartitionSpec("core"),) * (n_params + n_outs),
                      out_specs=(PartitionSpec("core"),) * n_outs,
                      check_rep=False),
            donate_argnums=tuple(range(n_params, n_params + n_outs)),
            keep_unused=True,
        )
        _CACHE["pjrt"] = (sharded, in_names, out_names, out_avals, zero_shapes)

    sharded, in_names, out_names, out_avals, zero_shapes = _CACHE["pjrt"]
    n_cores_ax = len(in_maps)
    concat_in = [
        _np.concatenate([_np.asarray(in_maps[c][nm]) for c in range(n_cores_ax)], axis=0)
        for nm in in_names
    ]
    concat_zeros = [
        _np.zeros((n_cores_ax * s[0], *s[1:]), d) for s, d in zero_shapes
    ]
    out_arrs = sharded(*concat_in, *concat_zeros)
    return [
        {nm: _np.asarray(out_arrs[i]).reshape(n_cores_ax, *out_avals[i].shape)[c]
         for i, nm in enumerate(out_names)}
        for c in range(n_cores_ax)
    ]


def kernel(**inputs):
    nc = _build()
    in_maps = _prep_host(inputs)
    try:
        results = _run_cached_pjrt(nc, in_maps)
    except Exception:
        _CACHE.pop("pjrt", None)
        res = run_bass_kernel_spmd(nc, in_maps, core_ids=list(range(8)))
        _CACHE["last_results"] = res
        results = res.results
    feat = np.zeros((4, C, 64, 64), np.float32)
    o1 = np.zeros((4, C, 64, 64), np.float32)
    o2 = np.zeros((4, C, 64, 64), np.float32)
    for core in range(8):
        s, half = core // 2, core % 2
        r = results[core]
        feat[s, :, 32 * half:32 * half + 32] = r["feat"]
        o1[s, :, 32 * half:32 * half + 32] = r["o1"]
        o2[s, :, 32 * half:32 * half + 32] = r["o2"]
    return (feat, o1, o2)
